# revision 1
# baseline (speedup 1.0000x reference)
"""NT-Xent loss on 8 TRN2 NeuronCores.

Reference computes, for z = concat(z1, z2) (2N=8192 rows, D=256):
    zn  = z / max(||z||, eps)
    sim = (zn @ zn.T) / T, diag masked to -1e9
    loss = mean_i( logsumexp_j sim[i, j] - sim[i, pos(i)] ),  pos(i) = (i + N) % 2N

Sharding: 2N rows split into 8 blocks of 1024. Each core computes its
1024x8192 row-block of sim against the full replicated zn.T, fused with
exp+rowsum on ScalarE (accum_out), so the sim matrix is never
materialized in HBM.

SPMD trick: core c receives zn.T with columns rotated left by c*1024, so
on EVERY core its own rows sit at columns 0:1024 and the positive
partners at columns 4096:5120. All diagonal-window access patterns are
then compile-time constants, identical across cores; only the data
differs. The exp'd self-similarity and positive-pair diagonals are
extracted from the ScalarE output tile with an eye-mask multiply +
reduce on VectorE; the host subtracts exp(self/T) from the denominator
sum and computes log() and the final mean (tiny).
"""

import sys

if "/opt/trn_rl_repo" not in sys.path:
    sys.path.insert(0, "/opt/trn_rl_repo")

import ml_dtypes
import numpy as np

import concourse.bass as bass
import concourse.mybir as mybir
import concourse.tile as tile
from concourse import bacc
from concourse.bass_utils import run_bass_kernel_spmd

N = 4096
D = 256
TWO_N = 2 * N          # 8192
TEMPERATURE = 0.07
EPS = 1e-8
N_CORES = 8
ROWS_PER_CORE = TWO_N // N_CORES   # 1024
M_TILES = ROWS_PER_CORE // 128     # 8 row-tiles of 128
CB = 2048                          # psum / column-block width
N_CB = TWO_N // CB                 # 4 column blocks
POS_CB = N // CB                   # column block holding the positives (2)

_cached = {}


def _build_bass(m_tiles=M_TILES):
    f32 = mybir.dt.float32
    bf16 = mybir.dt.bfloat16
    nc = bacc.Bacc("TRN2", target_bir_lowering=False, debug=False)

    znt = nc.declare_dram_parameter("znt", [D, TWO_N], bf16, isOutput=False)
    eye = nc.declare_dram_parameter("eye", [128, 128], f32, isOutput=False)
    s_out = nc.declare_dram_parameter("S", [128, m_tiles], f32, isOutput=True)
    sexp_out = nc.declare_dram_parameter("sexp", [128, m_tiles], f32, isOutput=True)
    pexp_out = nc.declare_dram_parameter("pexp", [128, m_tiles], f32, isOutput=True)

    with tile.TileContext(nc) as tc:
        with (
            tc.tile_pool(name="zchunks", bufs=1) as zpool,
            tc.tile_pool(name="consts", bufs=1) as cpool,
            tc.tile_pool(name="stats", bufs=1) as spool,
            tc.tile_pool(name="discard", bufs=4) as dpool,
            tc.tile_pool(name="scratch", bufs=2) as scpool,
            tc.tile_pool(name="psum", bufs=2, space=bass.MemorySpace.PSUM) as ppool,
        ):
            # Column-chunked copies of znt: zt[k][cb] holds rows k*128:(k+1)*128,
            # cols cb*2048:(cb+1)*2048. Separate tiles => independent DMA deps,
            # so phase cb only waits on its own chunks.
            zt = [[None] * N_CB for _ in range(2)]
            for cb in range(N_CB):
                for k in range(2):
                    t = zpool.tile([128, CB], bf16, tag=f"z{k}_{cb}")
                    # alternate DMA issue engines so descriptor issue isn't
                    # serialized on one queue (head-latency win)
                    eng = nc.sync if (cb * 2 + k) % 2 == 0 else nc.gpsimd
                    eng.dma_start(t[:], znt[k * 128 : (k + 1) * 128, cb * CB : (cb + 1) * CB])
                    zt[k][cb] = t

            eye_t = cpool.tile([128, 128], f32, tag="eye")
            nc.sync.dma_start(eye_t[:], eye[:])

            acc = spool.tile([128, m_tiles * N_CB], f32, tag="acc")
            s_t = spool.tile([128, m_tiles], f32, tag="S")
            sexp_t = spool.tile([128, m_tiles], f32, tag="sexp")
            pexp_t = spool.tile([128, m_tiles], f32, tag="pexp")

            for cb in range(N_CB):
                for m in range(m_tiles):
                    moff = m * 128
                    ps = ppool.tile([128, CB], f32, tag="ps")
                    for k in range(2):
                        for nn in range(CB // 512):
                            nc.tensor.matmul(
                                ps[:, nn * 512 : (nn + 1) * 512],
                                lhsT=zt[k][0][:, moff : moff + 128],
                                rhs=zt[k][cb][:, nn * 512 : (nn + 1) * 512],
                                start=(k == 0),
                                stop=(k == 1),
                            )
                    ex = dpool.tile([128, CB], f32, tag="ex")
                    nc.scalar.activation(
                        out=ex[:],
                        in_=ps[:],
                        func=mybir.ActivationFunctionType.Exp,
                        bias=0.0,
                        scale=1.0 / TEMPERATURE,
                        accum_out=acc[:, m * N_CB + cb : m * N_CB + cb + 1],
                    )
                    # extract exp'd diagonals from the SBUF exp tile:
                    # self-sim diag lives in cb 0 at cols moff:moff+128,
                    # positive-pair diag in cb POS_CB at the same offset.
                    for cond, dst in ((cb == 0, sexp_t), (cb == POS_CB, pexp_t)):
                        if cond:
                            poff = moff  # 4096 % CB == 0: same offset in cb 0 and cb 2
                            sc = scpool.tile([128, 128], f32, tag="sc")
                            nc.vector.tensor_tensor(
                                sc[:],
                                ex[:, poff : poff + 128],
                                eye_t[:],
                                mybir.AluOpType.mult,
                            )
                            nc.vector.reduce_sum(
                                dst[:, m : m + 1], sc[:], axis=mybir.AxisListType.X
                            )
                    if cb == N_CB - 1:
                        # final rowsum for this row-tile as soon as its last
                        # column block is done — overlaps the kernel tail
                        nc.vector.reduce_sum(
                            s_t[:, m : m + 1],
                            acc[:, m * N_CB : (m + 1) * N_CB],
                            axis=mybir.AxisListType.X,
                        )

            nc.sync.dma_start(s_out[:], s_t[:])
            nc.sync.dma_start(sexp_out[:], sexp_t[:])
            nc.sync.dma_start(pexp_out[:], pexp_t[:])

    nc.compile()
    return nc


def _prepare_inputs(z1, z2):
    z = np.concatenate([np.asarray(z1), np.asarray(z2)], axis=0).astype(np.float32)
    norms = np.maximum(np.sqrt((z.astype(np.float64) ** 2).sum(-1)), EPS)
    zn = (z / norms[:, None]).astype(np.float32)
    znb = zn.astype(ml_dtypes.bfloat16)
    znt = np.ascontiguousarray(znb.T)  # [D, 2N]
    eye = np.eye(128, dtype=np.float32)
    in_maps = []
    for c in range(N_CORES):
        znt_c = np.ascontiguousarray(np.roll(znt, -c * ROWS_PER_CORE, axis=1))
        in_maps.append({"znt": znt_c, "eye": eye})
    return in_maps


def kernel(z1, z2):
    if "nc" not in _cached:
        _cached["nc"] = _build_bass()
    nc = _cached["nc"]
    in_maps = _prepare_inputs(z1, z2)
    res = run_bass_kernel_spmd(nc, in_maps, core_ids=list(range(N_CORES)))
    results = res.results

    per_row_loss = np.zeros(TWO_N, dtype=np.float64)
    for c in range(N_CORES):
        # [128, M_TILES]; element [l, m] belongs to global row c*1024 + m*128 + l
        S = np.asarray(results[c]["S"], dtype=np.float64)
        sexp = np.asarray(results[c]["sexp"], dtype=np.float64)
        pexp = np.asarray(results[c]["pexp"], dtype=np.float64)
        # drop the self-similarity term from the softmax denominator, then
        # loss_i = log(sum_{j!=i} exp(sim/T)) - pos/T
        rows = np.log((S - sexp).T.reshape(-1)) - np.log(pexp.T.reshape(-1))
        per_row_loss[c * ROWS_PER_CORE : (c + 1) * ROWS_PER_CORE] = rows
    return np.float32(per_row_loss.mean())



# revision 3
# speedup vs baseline: 1.3523x; 1.3523x over previous
"""NT-Xent loss on 8 TRN2 NeuronCores — symmetric fp8 version.

Reference computes, for z = concat(z1, z2) (2N=8192 rows, D=256):
    zn  = z / max(||z||, eps)
    sim = (zn @ zn.T) / T, diag masked to -1e9
    loss = mean_i( logsumexp_j sim[i, j] - sim[i, pos(i)] ),  pos(i) = (i + N) % 2N

Strategy vs the plain row-sharded version:
  * sim is SYMMETRIC, so each core only computes column-strips at block
    distance 0..8 of its own rows (4608 of 8192 columns per row); the
    transposed contributions (distance 9..15) are recovered as COLUMN
    sums of the exp'd tiles from the cores that did compute them.
    This cuts the ScalarE exp work (the kernel bottleneck) by ~45%.
  * Matmuls run in fp8e4 DoubleRow perf mode: zn is scaled by 16,
    quantized to fp8, and laid out in [128, 2, cols] contract-pairs, so
    the full 256-deep contraction is a single PE pass at 2 MACs/cell.
    This also halves the HBM input traffic (the DMA head).
  * The self-similarity diagonal is killed inside PSUM by one extra
    accumulating matmul (16*I)^T @ (-240*I) = -3840*I, which drives
    exp() to 0 exactly — no host-side cancellation needed.
  * Column sums accumulate across the whole kernel into a single
    [16, 512] PSUM bank via tiny selector-weight matmuls (float32r).
  * Positive-pair logits are the diagonal of the distance-8 strip,
    extracted with an eye-mask multiply + reduce on VectorE.

Layout per core (SPMD; data rotated by c*1024 columns on the host so all
cores run the identical program):
  rows: 1024 (8 m-tiles of 128). 16 global col-strips of 512.
  m-tiles 0-3 (row-strip 0) read col-strips 0..8  (abs cols [0, 4608)),
  m-tiles 4-7 (row-strip 1) read col-strips 1..9  (abs cols [512, 5120)).
  Per m: 3 PSUM tiles [128, 1536] -> 3 exp+rowsum ACTIVATEs.
  Col-strip k of core c holds global rows ((2c+k) % 16) * 512 + j.
Host combines rowsums + colsums + pexp into the scalar loss in float64.
"""

import sys

if "/opt/trn_rl_repo" not in sys.path:
    sys.path.insert(0, "/opt/trn_rl_repo")

import ml_dtypes
import numpy as np

import concourse.bass as bass
import concourse.mybir as mybir
import concourse.tile as tile
from concourse import bacc
from concourse.bass_utils import run_bass_kernel_spmd

N = 4096
D = 256
TWO_N = 2 * N
TEMPERATURE = 0.07
EPS = 1e-8
N_CORES = 8
ROWS_PER_CORE = TWO_N // N_CORES   # 1024
M_TILES = ROWS_PER_CORE // 128     # 8
FP8_SCALE = 16.0                   # zn pre-scale before fp8 quantization
ACT_SCALE = 1.0 / (TEMPERATURE * FP8_SCALE * FP8_SCALE)
STRIP = 512                        # column strip width
WINDOW = 9 * STRIP                 # 4608 columns of exp work per row
SPAN = 10 * STRIP                  # 5120 columns of znt needed per core
QT = 1536                          # act/psum tile width (3 per m-tile)

_cached = {}


def _build_bass():
    f32 = mybir.dt.float32
    bf16 = mybir.dt.bfloat16
    fp8 = mybir.dt.float8e4
    DR = mybir.MatmulPerfMode.DoubleRow
    nc = bacc.Bacc("TRN2", target_bir_lowering=False, debug=False)

    # inputs
    znt = nc.declare_dram_parameter("znt", [128, 2, SPAN], fp8, isOutput=False)
    eyepair = nc.declare_dram_parameter("eyepair", [128, 256], fp8, isOutput=False)
    eye = nc.declare_dram_parameter("eye", [128, 128], bf16, isOutput=False)
    sel = nc.declare_dram_parameter("sel", [128, 128], bf16, isOutput=False)
    # outputs
    acc_out = nc.declare_dram_parameter("acc", [128, 3 * M_TILES], f32, isOutput=True)
    pexp_out = nc.declare_dram_parameter("pexp", [128, M_TILES], f32, isOutput=True)
    cs_out = nc.declare_dram_parameter("cs", [16, STRIP], f32, isOutput=True)

    with tile.TileContext(nc) as tc:
        with (
            tc.tile_pool(name="zchunks", bufs=1) as zpool,
            tc.tile_pool(name="consts", bufs=1) as cpool,
            tc.tile_pool(name="stats", bufs=1) as spool,
            tc.tile_pool(name="ex", bufs=4) as expool,
            tc.tile_pool(name="scratch", bufs=2) as scpool,
            tc.tile_pool(name="psum", bufs=2, space=bass.MemorySpace.PSUM) as ppool,
            tc.tile_pool(name="cspsum", bufs=1, space=bass.MemorySpace.PSUM) as cspool,
        ):
            # znt in 5 column chunks of 1024 so compute only waits on the
            # chunks it needs; alternate DMA issue queues.
            CHUNK = 1024
            zt = []
            for ci in range(SPAN // CHUNK):
                t = zpool.tile([128, 2, CHUNK], fp8, tag=f"z{ci}")
                eng = nc.sync if ci % 2 == 0 else nc.gpsimd
                eng.dma_start(t[:, :, :], znt[:, :, ci * CHUNK:(ci + 1) * CHUNK])
                zt.append(t)

            eyepair_t = cpool.tile([128, 256], fp8, tag="eyepair")
            nc.gpsimd.dma_start(eyepair_t[:], eyepair[:])
            eye_t = cpool.tile([128, 128], bf16, tag="eye")
            nc.sync.dma_start(eye_t[:], eye[:])
            sel_t = cpool.tile([128, 128], bf16, tag="sel")
            nc.gpsimd.dma_start(sel_t[:], sel[:])

            acc_t = spool.tile([128, 3 * M_TILES], f32, tag="acc")
            pexp_t = spool.tile([128, M_TILES], f32, tag="pexp")
            cs_ps = cspool.tile([16, STRIP], f32, tag="cs")
            cs_sb = spool.tile([16, STRIP], f32, tag="cs_sb")

            def chunk_slice(abs_col, width):
                ci, rel = abs_col // CHUNK, abs_col % CHUNK
                return zt[ci][:, :, rel:rel + width]

            for m in range(M_TILES):
                s = m // 4
                woff = STRIP * s
                exs = []
                for q in range(3):
                    ps = ppool.tile([128, QT], f32, tag="ps")
                    for j in range(3):
                        abs_col = woff + QT * q + 512 * j
                        first_bank = (q == 0 and j == 0)
                        nc.tensor.matmul(
                            ps[:, 512 * j:512 * j + 512],
                            lhsT=chunk_slice(128 * m, 128),
                            rhs=chunk_slice(abs_col, 512),
                            start=True,
                            stop=not first_bank,
                            perf_mode=DR,
                        )
                    if q == 0:
                        # kill the self-similarity diagonal: += -3840*I
                        rel_dk = 128 * m - woff  # always in [0, 512)
                        nc.tensor.matmul(
                            ps[:, rel_dk:rel_dk + 128],
                            lhsT=eyepair_t[:, 0:128],
                            rhs=eyepair_t[:, 128:256],
                            start=False,
                            stop=True,
                        )
                    ex = expool.tile([128, QT], bf16, tag="ex")
                    nc.scalar.activation(
                        out=ex[:],
                        in_=ps[:],
                        func=mybir.ActivationFunctionType.Exp,
                        bias=0.0,
                        scale=ACT_SCALE,
                        accum_out=acc_t[:, 3 * m + q:3 * m + q + 1],
                    )
                    exs.append(ex)

                # positive-pair logits: diagonal of the distance-8 strip,
                # at ex[2] cols [1024 + 128*(m%4), +128), diag offset p.
                poff = 1024 + 128 * (m % 4)
                sc = scpool.tile([128, 128], f32, tag="sc")
                nc.vector.tensor_tensor(
                    sc[:], exs[2][:, poff:poff + 128], eye_t[:],
                    mybir.AluOpType.mult,
                )
                nc.vector.reduce_sum(
                    pexp_t[:, m:m + 1], sc[:], axis=mybir.AxisListType.X
                )

                # column sums of distance 1..7 strips (window-rel cols
                # [512, 4096)) into cs_ps row k = local col-strip index.
                for d in range(1, 8):
                    k = d + s
                    wrel = 512 * d
                    q_idx, rel = wrel // QT, wrel % QT
                    nc.tensor.matmul(
                        cs_ps[:, :],
                        lhsT=sel_t[:, 16 * (k - 1):16 * k],
                        rhs=exs[q_idx][:, rel:rel + 512],
                        start=(m == 0 and d == 1),
                        stop=(m == M_TILES - 1 and d == 7),
                    )

            nc.vector.tensor_copy(cs_sb[:], cs_ps[:])
            nc.sync.dma_start(acc_out[:], acc_t[:])
            nc.sync.dma_start(pexp_out[:], pexp_t[:])
            nc.sync.dma_start(cs_out[:], cs_sb[:])

    nc.compile()
    return nc


def _prepare_inputs(z1, z2):
    z = np.concatenate([np.asarray(z1), np.asarray(z2)], axis=0).astype(np.float32)
    norms = np.maximum(np.sqrt((z.astype(np.float64) ** 2).sum(-1)), EPS)
    zn = (z / norms[:, None]).astype(np.float32)
    q = np.clip(zn * FP8_SCALE, -240.0, 240.0).astype(ml_dtypes.float8_e4m3)
    # paired layout: znt_p[p, i, j] = q[j, 128*i + p]
    znt_p = np.ascontiguousarray(q.T.reshape(2, 128, TWO_N).transpose(1, 0, 2))

    ey = np.eye(128, dtype=np.float32)
    eyepair = np.concatenate(
        [(16.0 * ey), (-240.0 * ey)], axis=1
    ).astype(ml_dtypes.float8_e4m3)
    ey = ey.astype(ml_dtypes.bfloat16)
    sel = np.zeros((128, 128), dtype=ml_dtypes.bfloat16)
    for k in range(1, 9):
        sel[:, 16 * (k - 1) + k] = 1.0

    in_maps = []
    for c in range(N_CORES):
        znt_c = np.ascontiguousarray(
            np.roll(znt_p, -c * ROWS_PER_CORE, axis=2)[:, :, :SPAN]
        )
        in_maps.append({"znt": znt_c, "eyepair": eyepair, "eye": ey, "sel": sel})
    return in_maps


def kernel(z1, z2):
    if "nc" not in _cached:
        _cached["nc"] = _build_bass()
    nc = _cached["nc"]
    in_maps = _prepare_inputs(z1, z2)
    res = run_bass_kernel_spmd(nc, in_maps, core_ids=list(range(N_CORES)))
    results = res.results

    denom = np.zeros(TWO_N, dtype=np.float64)
    pexp = np.zeros(TWO_N, dtype=np.float64)
    for c in range(N_CORES):
        acc = np.asarray(results[c]["acc"], dtype=np.float64)    # [128, 24]
        px = np.asarray(results[c]["pexp"], dtype=np.float64)    # [128, 8]
        cs = np.asarray(results[c]["cs"], dtype=np.float64)      # [16, 512]
        rows = slice(c * ROWS_PER_CORE, (c + 1) * ROWS_PER_CORE)
        # row r = 128*m + p  ->  acc[p, 3m:3m+3].sum()
        denom[rows] += acc.reshape(128, M_TILES, 3).sum(-1).T.reshape(-1)
        pexp[rows] = px.T.reshape(-1)
        for k in range(1, 9):
            g0 = ((2 * c + k) % 16) * STRIP
            denom[g0:g0 + STRIP] += cs[k]
    loss_rows = np.log(denom) - np.log(pexp)
    return np.float32(loss_rows.mean())


# revision 6
# speedup vs baseline: 1.4096x; 1.0423x over previous
"""NT-Xent loss on 8 TRN2 NeuronCores — symmetric fp8 version.

Reference computes, for z = concat(z1, z2) (2N=8192 rows, D=256):
    zn  = z / max(||z||, eps)
    sim = (zn @ zn.T) / T, diag masked to -1e9
    loss = mean_i( logsumexp_j sim[i, j] - sim[i, pos(i)] ),  pos(i) = (i + N) % 2N

Strategy vs the plain row-sharded version:
  * sim is SYMMETRIC, so each core only computes column-strips at block
    distance 0..8 of its own rows (4608 of 8192 columns per row); the
    transposed contributions (distance 9..15) are recovered as COLUMN
    sums of the exp'd tiles from the cores that did compute them.
    This cuts the ScalarE exp work (the kernel bottleneck) by ~45%.
  * Matmuls run in fp8e4 DoubleRow perf mode: zn is scaled by 16,
    quantized to fp8, and laid out in [128, 2, cols] contract-pairs, so
    the full 256-deep contraction is a single PE pass at 2 MACs/cell.
    This also halves the HBM input traffic (the DMA head).
  * The self-similarity diagonal is killed inside PSUM by one extra
    accumulating matmul (16*I)^T @ (-240*I) = -3840*I, which drives
    exp() to 0 exactly — no host-side cancellation needed.
  * Column sums accumulate across the whole kernel into a single
    [16, 512] PSUM bank via tiny selector-weight matmuls (float32r).
  * Positive-pair logits are the diagonal of the distance-8 strip,
    extracted with an eye-mask multiply + reduce on VectorE.

Layout per core (SPMD; data rotated by c*1024 columns on the host so all
cores run the identical program):
  rows: 1024 (8 m-tiles of 128). 16 global col-strips of 512.
  m-tiles 0-3 (row-strip 0) read col-strips 0..8  (abs cols [0, 4608)),
  m-tiles 4-7 (row-strip 1) read col-strips 1..9  (abs cols [512, 5120)).
  Per m: 3 PSUM tiles [128, 1536] -> 3 exp+rowsum ACTIVATEs.
  Col-strip k of core c holds global rows ((2c+k) % 16) * 512 + j.
Host combines rowsums + colsums + pexp into the scalar loss in float64.
"""

import sys

if "/opt/trn_rl_repo" not in sys.path:
    sys.path.insert(0, "/opt/trn_rl_repo")

import ml_dtypes
import numpy as np

import concourse.bass as bass
import concourse.mybir as mybir
import concourse.tile as tile
from concourse import bacc
from concourse.bass_utils import run_bass_kernel_spmd

N = 4096
D = 256
TWO_N = 2 * N
TEMPERATURE = 0.07
EPS = 1e-8
N_CORES = 8
ROWS_PER_CORE = TWO_N // N_CORES   # 1024
M_TILES = ROWS_PER_CORE // 128     # 8
FP8_SCALE = 16.0                   # zn pre-scale before fp8 quantization
ACT_SCALE = 1.0 / (TEMPERATURE * FP8_SCALE * FP8_SCALE)
STRIP = 512                        # column strip width
WINDOW = 9 * STRIP                 # 4608 columns of exp work per row
SPAN = 10 * STRIP                  # 5120 columns of znt needed per core
QT = 1536                          # act/psum tile width (3 per m-tile)

_cached = {}


def _build_bass():
    f32 = mybir.dt.float32
    bf16 = mybir.dt.bfloat16
    fp8 = mybir.dt.float8e4
    DR = mybir.MatmulPerfMode.DoubleRow
    nc = bacc.Bacc("TRN2", target_bir_lowering=False, debug=False)

    # inputs
    znt = nc.declare_dram_parameter("znt", [128, 2, SPAN], fp8, isOutput=False)
    eyepair = nc.declare_dram_parameter("eyepair", [128, 256], fp8, isOutput=False)
    eye = nc.declare_dram_parameter("eye", [128, 128], bf16, isOutput=False)
    sel = nc.declare_dram_parameter("sel", [128, 128], bf16, isOutput=False)
    # outputs
    acc_out = nc.declare_dram_parameter("acc", [128, 3 * M_TILES], f32, isOutput=True)
    pexp_out = nc.declare_dram_parameter("pexp", [128, M_TILES], f32, isOutput=True)
    cs_out = nc.declare_dram_parameter("cs", [16, STRIP], f32, isOutput=True)

    with tile.TileContext(nc) as tc:
        with (
            tc.tile_pool(name="zchunks", bufs=1) as zpool,
            tc.tile_pool(name="consts", bufs=1) as cpool,
            tc.tile_pool(name="stats", bufs=1) as spool,
            tc.tile_pool(name="ex", bufs=4) as expool,
            tc.tile_pool(name="scratch", bufs=2) as scpool,
            tc.tile_pool(name="psum", bufs=2, space=bass.MemorySpace.PSUM) as ppool,
            tc.tile_pool(name="cspsum", bufs=1, space=bass.MemorySpace.PSUM) as cspool,
        ):
            # Tiny consts first (the diag-kill matmul needs eyepair before
            # the first act); znt in 5 column chunks of 1024 so compute
            # only waits on the chunks it needs, split across both queues
            # in consumption order.
            eyepair_t = cpool.tile([128, 256], fp8, tag="eyepair")
            nc.gpsimd.dma_start(eyepair_t[:], eyepair[:])
            eye_t = cpool.tile([128, 128], bf16, tag="eye")
            nc.gpsimd.dma_start(eye_t[:], eye[:])
            sel_t = cpool.tile([128, 128], bf16, tag="sel")
            nc.gpsimd.dma_start(sel_t[:], sel[:])

            CHUNK = 1024
            zt = []
            for ci in range(SPAN // CHUNK):
                t = zpool.tile([128, 2, CHUNK], fp8, tag=f"z{ci}")
                eng = nc.sync if ci % 2 == 0 else nc.gpsimd
                eng.dma_start(t[:, :, :], znt[:, :, ci * CHUNK:(ci + 1) * CHUNK])
                zt.append(t)

            acc_t = spool.tile([128, 3 * M_TILES], f32, tag="acc")
            pexp_t = spool.tile([128, M_TILES], f32, tag="pexp")
            cs_ps = cspool.tile([16, STRIP], f32, tag="cs")
            cs_sb = spool.tile([16, STRIP], f32, tag="cs_sb")

            def chunk_slice(abs_col, width):
                ci, rel = abs_col // CHUNK, abs_col % CHUNK
                return zt[ci][:, :, rel:rel + width]

            # q emission order 0,2,1: the pexp diag and the d6/d7 colsum
            # segments live in q2, so finishing q2 before q1 lets the
            # m=7 tail overlap the last ACTIVATE.
            Q_ORDER = (0, 2, 1)
            # colsum strip d -> (q tile, rel offset): emitted grouped by
            # the q-tile that produces the segment, in Q_ORDER.
            CS_BY_Q = {0: (1, 2), 2: (6, 7), 1: (3, 4, 5)}

            for m in range(M_TILES):
                s = m // 4
                woff = STRIP * s
                exs = {}
                for q in Q_ORDER:
                    ps = ppool.tile([128, QT], f32, tag="ps")
                    for j in range(3):
                        abs_col = woff + QT * q + 512 * j
                        first_bank = (q == 0 and j == 0)
                        nc.tensor.matmul(
                            ps[:, 512 * j:512 * j + 512],
                            lhsT=chunk_slice(128 * m, 128),
                            rhs=chunk_slice(abs_col, 512),
                            start=True,
                            stop=not first_bank,
                            perf_mode=DR,
                        )
                    if q == 0:
                        # kill the self-similarity diagonal: += -3840*I
                        rel_dk = 128 * m - woff  # always in [0, 512)
                        nc.tensor.matmul(
                            ps[:, rel_dk:rel_dk + 128],
                            lhsT=eyepair_t[:, 0:128],
                            rhs=eyepair_t[:, 128:256],
                            start=False,
                            stop=True,
                        )
                    ex = expool.tile([128, QT], bf16, tag="ex")
                    nc.scalar.activation(
                        out=ex[:],
                        in_=ps[:],
                        func=mybir.ActivationFunctionType.Exp,
                        bias=0.0,
                        scale=ACT_SCALE,
                        accum_out=acc_t[:, 3 * m + q:3 * m + q + 1],
                    )
                    exs[q] = ex

                    if q == 2:
                        # positive-pair logits: diagonal of the distance-8
                        # strip, at ex cols [1024 + 128*(m%4), +128).
                        poff = 1024 + 128 * (m % 4)
                        sc = scpool.tile([128, 128], f32, tag="sc")
                        nc.vector.tensor_tensor(
                            sc[:], ex[:, poff:poff + 128], eye_t[:],
                            mybir.AluOpType.mult,
                        )
                        nc.vector.reduce_sum(
                            pexp_t[:, m:m + 1], sc[:], axis=mybir.AxisListType.X
                        )

                # column sums of distance 1..7 strips (window-rel cols
                # [512, 4096)) into cs_ps row k = local col-strip index,
                # emitted in ex-readiness order.
                first_cs = (m == 0)
                cs_list = [d for q in Q_ORDER for d in CS_BY_Q[q]]
                for i, d in enumerate(cs_list):
                    k = d + s
                    wrel = 512 * d
                    q_idx, rel = wrel // QT, wrel % QT
                    nc.tensor.matmul(
                        cs_ps[:, :],
                        lhsT=sel_t[:, 16 * (k - 1):16 * k],
                        rhs=exs[q_idx][:, rel:rel + 512],
                        start=(first_cs and i == 0),
                        stop=(m == M_TILES - 1 and i == len(cs_list) - 1),
                    )

            nc.vector.tensor_copy(cs_sb[:], cs_ps[:])
            nc.sync.dma_start(acc_out[:], acc_t[:])
            nc.gpsimd.dma_start(pexp_out[:], pexp_t[:])
            nc.sync.dma_start(cs_out[:], cs_sb[:])

    nc.compile()
    return nc


def _prepare_inputs(z1, z2):
    z = np.concatenate([np.asarray(z1), np.asarray(z2)], axis=0).astype(np.float32)
    norms = np.maximum(np.sqrt((z.astype(np.float64) ** 2).sum(-1)), EPS)
    zn = (z / norms[:, None]).astype(np.float32)
    q = np.clip(zn * FP8_SCALE, -240.0, 240.0).astype(ml_dtypes.float8_e4m3)
    # paired layout: znt_p[p, i, j] = q[j, 128*i + p]
    znt_p = np.ascontiguousarray(q.T.reshape(2, 128, TWO_N).transpose(1, 0, 2))

    ey = np.eye(128, dtype=np.float32)
    eyepair = np.concatenate(
        [(16.0 * ey), (-240.0 * ey)], axis=1
    ).astype(ml_dtypes.float8_e4m3)
    ey = ey.astype(ml_dtypes.bfloat16)
    sel = np.zeros((128, 128), dtype=ml_dtypes.bfloat16)
    for k in range(1, 9):
        sel[:, 16 * (k - 1) + k] = 1.0

    in_maps = []
    for c in range(N_CORES):
        znt_c = np.ascontiguousarray(
            np.roll(znt_p, -c * ROWS_PER_CORE, axis=2)[:, :, :SPAN]
        )
        in_maps.append({"znt": znt_c, "eyepair": eyepair, "eye": ey, "sel": sel})
    return in_maps


def kernel(z1, z2):
    if "nc" not in _cached:
        _cached["nc"] = _build_bass()
    nc = _cached["nc"]
    in_maps = _prepare_inputs(z1, z2)
    res = run_bass_kernel_spmd(nc, in_maps, core_ids=list(range(N_CORES)))
    results = res.results

    denom = np.zeros(TWO_N, dtype=np.float64)
    pexp = np.zeros(TWO_N, dtype=np.float64)
    for c in range(N_CORES):
        acc = np.asarray(results[c]["acc"], dtype=np.float64)    # [128, 24]
        px = np.asarray(results[c]["pexp"], dtype=np.float64)    # [128, 8]
        cs = np.asarray(results[c]["cs"], dtype=np.float64)      # [16, 512]
        rows = slice(c * ROWS_PER_CORE, (c + 1) * ROWS_PER_CORE)
        # row r = 128*m + p  ->  acc[p, 3m:3m+3].sum()
        denom[rows] += acc.reshape(128, M_TILES, 3).sum(-1).T.reshape(-1)
        pexp[rows] = px.T.reshape(-1)
        for k in range(1, 9):
            g0 = ((2 * c + k) % 16) * STRIP
            denom[g0:g0 + STRIP] += cs[k]
    loss_rows = np.log(denom) - np.log(pexp)
    return np.float32(loss_rows.mean())


# revision 9
# speedup vs baseline: 1.4278x; 1.0129x over previous
"""NT-Xent loss on 8 TRN2 NeuronCores — symmetric fp8 version.

Reference computes, for z = concat(z1, z2) (2N=8192 rows, D=256):
    zn  = z / max(||z||, eps)
    sim = (zn @ zn.T) / T, diag masked to -1e9
    loss = mean_i( logsumexp_j sim[i, j] - sim[i, pos(i)] ),  pos(i) = (i + N) % 2N

Strategy vs the plain row-sharded version:
  * sim is SYMMETRIC, so each core only computes column-strips at block
    distance 0..8 of its own rows (4608 of 8192 columns per row); the
    transposed contributions (distance 9..15) are recovered as COLUMN
    sums of the exp'd tiles from the cores that did compute them.
    This cuts the ScalarE exp work (the kernel bottleneck) by ~45%.
  * Matmuls run in fp8e4 DoubleRow perf mode: zn is scaled by 16,
    quantized to fp8, and laid out in [128, 2, cols] contract-pairs, so
    the full 256-deep contraction is a single PE pass at 2 MACs/cell.
    This also halves the HBM input traffic (the DMA head).
  * The self-similarity diagonal is killed inside PSUM by one extra
    accumulating matmul (16*I)^T @ (-240*I) = -3840*I, which drives
    exp() to 0 exactly — no host-side cancellation needed.
  * Column sums accumulate across the whole kernel into a single
    [16, 512] PSUM bank via tiny selector-weight matmuls (float32r).
  * Positive-pair logits are the diagonal of the distance-8 strip,
    extracted with an eye-mask multiply + reduce on VectorE.

Layout per core (SPMD; data rotated by c*1024 columns on the host so all
cores run the identical program):
  rows: 1024 (8 m-tiles of 128). 16 global col-strips of 512.
  m-tiles 0-3 (row-strip 0) read col-strips 0..8  (abs cols [0, 4608)),
  m-tiles 4-7 (row-strip 1) read col-strips 1..9  (abs cols [512, 5120)).
  Per m: 3 PSUM tiles [128, 1536] -> 3 exp+rowsum ACTIVATEs.
  Col-strip k of core c holds global rows ((2c+k) % 16) * 512 + j.
Host combines rowsums + colsums + pexp into the scalar loss in float64.
"""

import sys

if "/opt/trn_rl_repo" not in sys.path:
    sys.path.insert(0, "/opt/trn_rl_repo")

import ml_dtypes
import numpy as np

import concourse.bass as bass
import concourse.mybir as mybir
import concourse.tile as tile
from concourse import bacc
from concourse.bass_utils import run_bass_kernel_spmd

N = 4096
D = 256
TWO_N = 2 * N
TEMPERATURE = 0.07
EPS = 1e-8
N_CORES = 8
ROWS_PER_CORE = TWO_N // N_CORES   # 1024
M_TILES = ROWS_PER_CORE // 128     # 8
FP8_SCALE = 16.0                   # zn pre-scale before fp8 quantization
ACT_SCALE = 1.0 / (TEMPERATURE * FP8_SCALE * FP8_SCALE)
STRIP = 512                        # column strip width
WINDOW = 9 * STRIP                 # 4608 columns of exp work per row
SPAN = 10 * STRIP                  # 5120 columns of znt needed per core
QT = 1536                          # act/psum tile width (3 per m-tile)

_cached = {}


def _build_bass():
    f32 = mybir.dt.float32
    bf16 = mybir.dt.bfloat16
    fp8 = mybir.dt.float8e4
    DR = mybir.MatmulPerfMode.DoubleRow
    nc = bacc.Bacc("TRN2", target_bir_lowering=False, debug=False)

    # inputs
    znt = nc.declare_dram_parameter("znt", [128, 2, SPAN], fp8, isOutput=False)
    eyepair = nc.declare_dram_parameter("eyepair", [128, 256], fp8, isOutput=False)
    eye = nc.declare_dram_parameter("eye", [128, 128], bf16, isOutput=False)
    sel = nc.declare_dram_parameter("sel", [128, 128], bf16, isOutput=False)
    # outputs
    acc_out = nc.declare_dram_parameter("acc", [128, 3 * M_TILES], f32, isOutput=True)
    pexp_out = nc.declare_dram_parameter("pexp", [128, M_TILES], f32, isOutput=True)
    cs_out = nc.declare_dram_parameter("cs", [16, STRIP], f32, isOutput=True)

    with tile.TileContext(nc) as tc:
        with (
            tc.tile_pool(name="sb", bufs=1) as sb,
            tc.tile_pool(name="ps", bufs=1, space=bass.MemorySpace.PSUM) as pp,
        ):
            # znt chunks sized so the m=0 pipeline starts as early as
            # possible: q0 needs [0,1536), q1 [1536,3072), q2 [3072,4608).
            # Interleave across the two DMA queues in consumption order;
            # eyepair (needed by the q0 diag-kill) goes first on gpsimd.
            eyepair_t = sb.tile([128, 256], fp8, tag="eyepair")
            nc.gpsimd.dma_start(eyepair_t[:], eyepair[:])

            BOUNDS = [0, 512, 1536, 3072, 4608, SPAN]
            zt = []
            for ci in range(len(BOUNDS) - 1):
                c0, c1 = BOUNDS[ci], BOUNDS[ci + 1]
                t = sb.tile([128, 2, c1 - c0], fp8, tag=f"z{ci}")
                eng = nc.sync if ci % 2 == 0 else nc.gpsimd
                eng.dma_start(t[:, :, :], znt[:, :, c0:c1])
                zt.append(t)

            eye_t = sb.tile([128, 128], bf16, tag="eye")
            nc.sync.dma_start(eye_t[:], eye[:])
            sel_t = sb.tile([128, 128], bf16, tag="sel")
            nc.gpsimd.dma_start(sel_t[:], sel[:])

            acc_t = sb.tile([128, 3 * M_TILES], f32, tag="acc")
            pexp_t = sb.tile([128, M_TILES], f32, tag="pexp")
            cs_ps = pp.tile([16, STRIP], f32, tag="cs")
            cs_sb = sb.tile([16, STRIP], f32, tag="cs_sb")

            def chunk_slice(abs_col, width):
                for ci in range(len(BOUNDS) - 1):
                    if BOUNDS[ci] <= abs_col and abs_col + width <= BOUNDS[ci + 1]:
                        rel = abs_col - BOUNDS[ci]
                        return zt[ci][:, :, rel:rel + width]
                raise AssertionError(f"slice [{abs_col}, {abs_col + width}) crosses chunks")

            Q_ORDER = (0, 1, 2)
            # colsum strip d -> source q tile: d1,d2 in q0; d3-d5 in q1;
            # d6,d7 in q2. Emitted in that (readiness) order.
            CS_BY_Q = {0: (1, 2), 1: (3, 4, 5), 2: (6, 7)}

            for m in range(M_TILES):
                s = m // 4
                woff = STRIP * s
                exs = {}
                for q in Q_ORDER:
                    ps = pp.tile([128, QT], f32, tag="ps", bufs=2)
                    for j in range(3):
                        abs_col = woff + QT * q + 512 * j
                        first_bank = (q == 0 and j == 0)
                        nc.tensor.matmul(
                            ps[:, 512 * j:512 * j + 512],
                            lhsT=chunk_slice(128 * m, 128),
                            rhs=chunk_slice(abs_col, 512),
                            start=True,
                            stop=not first_bank,
                            perf_mode=DR,
                        )
                    if q == 0:
                        # kill the self-similarity diagonal: += -3840*I
                        rel_dk = 128 * m - woff  # always in [0, 512)
                        nc.tensor.matmul(
                            ps[:, rel_dk:rel_dk + 128],
                            lhsT=eyepair_t[:, 0:128],
                            rhs=eyepair_t[:, 128:256],
                            start=False,
                            stop=True,
                        )
                    ex = sb.tile([128, QT], bf16, tag="ex", bufs=4)
                    nc.scalar.activation(
                        out=ex[:],
                        in_=ps[:],
                        func=mybir.ActivationFunctionType.Exp,
                        bias=0.0,
                        scale=ACT_SCALE,
                        accum_out=acc_t[:, 3 * m + q:3 * m + q + 1],
                    )
                    exs[q] = ex

                    if q == 2:
                        # positive-pair logits: diagonal of the distance-8
                        # strip, at ex cols [1024 + 128*(m%4), +128).
                        poff = 1024 + 128 * (m % 4)
                        sc = sb.tile([128, 128], f32, tag="sc", bufs=2)
                        nc.vector.tensor_tensor(
                            sc[:], ex[:, poff:poff + 128], eye_t[:],
                            mybir.AluOpType.mult,
                        )
                        nc.vector.reduce_sum(
                            pexp_t[:, m:m + 1], sc[:], axis=mybir.AxisListType.X
                        )

                # column sums of distance 1..7 strips (window-rel cols
                # [512, 4096)) into cs_ps row k = local col-strip index,
                # emitted in ex-readiness order.
                first_cs = (m == 0)
                cs_list = [d for q in Q_ORDER for d in CS_BY_Q[q]]
                for i, d in enumerate(cs_list):
                    k = d + s
                    wrel = 512 * d
                    q_idx, rel = wrel // QT, wrel % QT
                    nc.tensor.matmul(
                        cs_ps[:, :],
                        lhsT=sel_t[:, 16 * (k - 1):16 * k],
                        rhs=exs[q_idx][:, rel:rel + 512],
                        start=(first_cs and i == 0),
                        stop=(m == M_TILES - 1 and i == len(cs_list) - 1),
                    )

            nc.vector.tensor_copy(cs_sb[:], cs_ps[:])
            nc.sync.dma_start(acc_out[:], acc_t[:])
            nc.gpsimd.dma_start(pexp_out[:], pexp_t[:])
            nc.sync.dma_start(cs_out[:], cs_sb[:])

    nc.compile()
    return nc


def _prepare_inputs(z1, z2):
    z = np.concatenate([np.asarray(z1), np.asarray(z2)], axis=0).astype(np.float32)
    norms = np.maximum(np.sqrt((z.astype(np.float64) ** 2).sum(-1)), EPS)
    zn = (z / norms[:, None]).astype(np.float32)
    q = np.clip(zn * FP8_SCALE, -240.0, 240.0).astype(ml_dtypes.float8_e4m3)
    # paired layout: znt_p[p, i, j] = q[j, 128*i + p]
    znt_p = np.ascontiguousarray(q.T.reshape(2, 128, TWO_N).transpose(1, 0, 2))

    ey = np.eye(128, dtype=np.float32)
    eyepair = np.concatenate(
        [(16.0 * ey), (-240.0 * ey)], axis=1
    ).astype(ml_dtypes.float8_e4m3)
    ey = ey.astype(ml_dtypes.bfloat16)
    sel = np.zeros((128, 128), dtype=ml_dtypes.bfloat16)
    for k in range(1, 9):
        sel[:, 16 * (k - 1) + k] = 1.0

    in_maps = []
    for c in range(N_CORES):
        znt_c = np.ascontiguousarray(
            np.roll(znt_p, -c * ROWS_PER_CORE, axis=2)[:, :, :SPAN]
        )
        in_maps.append({"znt": znt_c, "eyepair": eyepair, "eye": ey, "sel": sel})
    return in_maps


def kernel(z1, z2):
    if "nc" not in _cached:
        _cached["nc"] = _build_bass()
    nc = _cached["nc"]
    in_maps = _prepare_inputs(z1, z2)
    res = run_bass_kernel_spmd(nc, in_maps, core_ids=list(range(N_CORES)))
    results = res.results

    denom = np.zeros(TWO_N, dtype=np.float64)
    pexp = np.zeros(TWO_N, dtype=np.float64)
    for c in range(N_CORES):
        acc = np.asarray(results[c]["acc"], dtype=np.float64)    # [128, 24]
        px = np.asarray(results[c]["pexp"], dtype=np.float64)    # [128, 8]
        cs = np.asarray(results[c]["cs"], dtype=np.float64)      # [16, 512]
        rows = slice(c * ROWS_PER_CORE, (c + 1) * ROWS_PER_CORE)
        # row r = 128*m + p  ->  acc[p, 3m:3m+3].sum()
        denom[rows] += acc.reshape(128, M_TILES, 3).sum(-1).T.reshape(-1)
        pexp[rows] = px.T.reshape(-1)
        for k in range(1, 9):
            g0 = ((2 * c + k) % 16) * STRIP
            denom[g0:g0 + STRIP] += cs[k]
    loss_rows = np.log(denom) - np.log(pexp)
    return np.float32(loss_rows.mean())


# revision 12
# speedup vs baseline: 1.4423x; 1.0102x over previous
"""NT-Xent loss on 8 TRN2 NeuronCores — symmetric fp8 version.

Reference computes, for z = concat(z1, z2) (2N=8192 rows, D=256):
    zn  = z / max(||z||, eps)
    sim = (zn @ zn.T) / T, diag masked to -1e9
    loss = mean_i( logsumexp_j sim[i, j] - sim[i, pos(i)] ),  pos(i) = (i + N) % 2N

Strategy vs the plain row-sharded version:
  * sim is SYMMETRIC, so each core only computes column-strips at block
    distance 0..8 of its own rows (4608 of 8192 columns per row); the
    transposed contributions (distance 9..15) are recovered as COLUMN
    sums of the exp'd tiles from the cores that did compute them.
    This cuts the ScalarE exp work (the kernel bottleneck) by ~45%.
  * Matmuls run in fp8e4 DoubleRow perf mode: zn is scaled by 16,
    quantized to fp8, and laid out in [128, 2, cols] contract-pairs, so
    the full 256-deep contraction is a single PE pass at 2 MACs/cell.
    This also halves the HBM input traffic (the DMA head).
  * The self-similarity diagonal is killed inside PSUM by one extra
    accumulating matmul (16*I)^T @ (-240*I) = -3840*I, which drives
    exp() to 0 exactly — no host-side cancellation needed.
  * Column sums accumulate across the whole kernel into a single
    [16, 512] PSUM bank via tiny selector-weight matmuls (float32r).
  * Positive-pair logits are the diagonal of the distance-8 strip,
    extracted with an eye-mask multiply + reduce on VectorE.

Layout per core (SPMD; data rotated by c*1024 columns on the host so all
cores run the identical program):
  rows: 1024 (8 m-tiles of 128). 16 global col-strips of 512.
  m-tiles 0-3 (row-strip 0) read col-strips 0..8  (abs cols [0, 4608)),
  m-tiles 4-7 (row-strip 1) read col-strips 1..9  (abs cols [512, 5120)).
  Per m: 3 PSUM tiles [128, 1536] -> 3 exp+rowsum ACTIVATEs.
  Col-strip k of core c holds global rows ((2c+k) % 16) * 512 + j.
Host combines rowsums + colsums + pexp into the scalar loss in float64.
"""

import sys

if "/opt/trn_rl_repo" not in sys.path:
    sys.path.insert(0, "/opt/trn_rl_repo")

import ml_dtypes
import numpy as np

import concourse.bass as bass
import concourse.mybir as mybir
import concourse.tile as tile
from concourse import bacc
from concourse.bass_utils import run_bass_kernel_spmd

N = 4096
D = 256
TWO_N = 2 * N
TEMPERATURE = 0.07
EPS = 1e-8
N_CORES = 8
ROWS_PER_CORE = TWO_N // N_CORES   # 1024
M_TILES = ROWS_PER_CORE // 128     # 8
FP8_SCALE = 16.0                   # zn pre-scale before fp8 quantization
ACT_SCALE = 1.0 / (TEMPERATURE * FP8_SCALE * FP8_SCALE)
STRIP = 512                        # column strip width
WINDOW = 9 * STRIP                 # 4608 columns of exp work per row
SPAN = 10 * STRIP                  # 5120 columns of znt needed per core
QT = 1536                          # act/psum tile width (3 per m-tile)
N_ACTS = 3 * M_TILES + 1           # m=0 is split into 4 act tiles

_cached = {}


def _build_bass():
    f32 = mybir.dt.float32
    bf16 = mybir.dt.bfloat16
    fp8 = mybir.dt.float8e4
    DR = mybir.MatmulPerfMode.DoubleRow
    nc = bacc.Bacc("TRN2", target_bir_lowering=False, debug=False)

    # inputs
    znt = nc.declare_dram_parameter("znt", [128, 2, SPAN], fp8, isOutput=False)
    eyepair = nc.declare_dram_parameter("eyepair", [128, 256], fp8, isOutput=False)
    eye = nc.declare_dram_parameter("eye", [128, 128], bf16, isOutput=False)
    sel = nc.declare_dram_parameter("sel", [128, 128], bf16, isOutput=False)
    # outputs
    acc_out = nc.declare_dram_parameter("acc", [128, N_ACTS], f32, isOutput=True)
    pexp_out = nc.declare_dram_parameter("pexp", [128, M_TILES], f32, isOutput=True)
    cs_out = nc.declare_dram_parameter("cs", [16, STRIP], f32, isOutput=True)

    with tile.TileContext(nc) as tc:
        with (
            tc.tile_pool(name="sb", bufs=1) as sb,
            tc.tile_pool(name="ps", bufs=1, space=bass.MemorySpace.PSUM) as pp,
        ):
            # znt chunks sized so the m=0 pipeline starts as early as
            # possible: q0 needs [0,1536), q1 [1536,3072), q2 [3072,4608).
            # Interleave across the two DMA queues in consumption order;
            # eyepair (needed by the q0 diag-kill) goes first on gpsimd.
            eyepair_t = sb.tile([128, 256], fp8, tag="eyepair")
            nc.gpsimd.dma_start(eyepair_t[:], eyepair[:])

            BOUNDS = [0, 512, 1536, 3072, 4608, SPAN]
            zt = [None] * (len(BOUNDS) - 1)
            # queue assignment in m=0 consumption order: chunks 0,1 behind
            # each other on sync; 2,3 behind eyepair on gpsimd; 4 trails.
            for ci, eng in ((0, nc.sync), (1, nc.sync), (2, nc.gpsimd),
                            (3, nc.gpsimd), (4, nc.sync)):
                c0, c1 = BOUNDS[ci], BOUNDS[ci + 1]
                t = sb.tile([128, 2, c1 - c0], fp8, tag=f"z{ci}")
                eng.dma_start(t[:, :, :], znt[:, :, c0:c1])
                zt[ci] = t

            eye_t = sb.tile([128, 128], bf16, tag="eye")
            nc.sync.dma_start(eye_t[:], eye[:])
            sel_t = sb.tile([128, 128], bf16, tag="sel")
            nc.gpsimd.dma_start(sel_t[:], sel[:])

            acc_ps = pp.tile([128, N_ACTS], f32, tag="accps")
            acc_t = sb.tile([128, N_ACTS], f32, tag="acc")
            pexp_t = sb.tile([128, M_TILES], f32, tag="pexp")
            cs_ps = pp.tile([16, STRIP], f32, tag="cs")
            cs_sb = sb.tile([16, STRIP], f32, tag="cs_sb")

            def chunk_slice(abs_col, width):
                for ci in range(len(BOUNDS) - 1):
                    if BOUNDS[ci] <= abs_col and abs_col + width <= BOUNDS[ci + 1]:
                        rel = abs_col - BOUNDS[ci]
                        return zt[ci][:, :, rel:rel + width]
                raise AssertionError(f"slice [{abs_col}, {abs_col + width}) crosses chunks")

            aidx = 0
            for m in range(M_TILES):
                s = m // 4
                woff = STRIP * s
                # act/psum tile spans in window-absolute columns. m=0 is
                # split finer so the first ACTIVATEs only wait on the
                # first DMA chunks.
                if m == 0:
                    spans = [(0, 512), (512, 1536), (1536, 3072), (3072, 4608)]
                else:
                    spans = [(woff + QT * q, woff + QT * (q + 1)) for q in range(3)]

                def span_slice(abs_col, width):
                    for ti, (a, b) in enumerate(spans):
                        if a <= abs_col and abs_col + width <= b:
                            return ti, abs_col - a
                    raise AssertionError(f"[{abs_col},{abs_col + width}) not in spans")

                exs = []
                dk_tile, dk_rel = span_slice(128 * m, 128)
                for ti, (a, b) in enumerate(spans):
                    ps = pp.tile([128, QT], f32, tag="ps", bufs=2)
                    w = b - a
                    for j in range(w // 512):
                        abs_col = a + 512 * j
                        nc.tensor.matmul(
                            ps[:, 512 * j:512 * j + 512],
                            lhsT=chunk_slice(128 * m, 128),
                            rhs=chunk_slice(abs_col, 512),
                            start=True,
                            stop=not (ti == dk_tile and j == dk_rel // 512),
                            perf_mode=DR,
                        )
                    if ti == dk_tile:
                        # kill the self-similarity diagonal: += -3840*I
                        nc.tensor.matmul(
                            ps[:, dk_rel:dk_rel + 128],
                            lhsT=eyepair_t[:, 0:128],
                            rhs=eyepair_t[:, 128:256],
                            start=False,
                            stop=True,
                        )
                    ex = sb.tile([128, QT], bf16, tag="ex", bufs=4)
                    nc.scalar.activation(
                        out=ex[:, 0:w],
                        in_=ps[:, 0:w],
                        func=mybir.ActivationFunctionType.Exp,
                        bias=0.0,
                        scale=ACT_SCALE,
                        accum_out=acc_ps[:, aidx:aidx + 1],
                    )
                    aidx += 1
                    exs.append(ex)

                # positive-pair logits: diagonal of the distance-8 strip
                # (abs cols [woff+4096+128*(m%4), +128)), in the last tile.
                pti, prel = span_slice(woff + 4096 + 128 * (m % 4), 128)
                sc = sb.tile([128, 128], f32, tag="sc", bufs=2)
                nc.vector.tensor_tensor(
                    sc[:], exs[pti][:, prel:prel + 128], eye_t[:],
                    mybir.AluOpType.mult,
                )
                nc.vector.reduce_sum(
                    pexp_t[:, m:m + 1], sc[:], axis=mybir.AxisListType.X
                )

                # column sums of distance 1..7 strips into cs_ps row
                # k = local col-strip index, in ex-readiness order.
                for i, d in enumerate(range(1, 8)):
                    k = d + s
                    ti, rel = span_slice(woff + 512 * d, 512)
                    nc.tensor.matmul(
                        cs_ps[:, :],
                        lhsT=sel_t[:, 16 * (k - 1):16 * k],
                        rhs=exs[ti][:, rel:rel + 512],
                        start=(m == 0 and i == 0),
                        stop=(m == M_TILES - 1 and i == 6),
                    )

            nc.vector.tensor_copy(acc_t[:], acc_ps[:])
            nc.vector.tensor_copy(cs_sb[:], cs_ps[:])
            nc.sync.dma_start(acc_out[:], acc_t[:])
            nc.gpsimd.dma_start(pexp_out[:], pexp_t[:])
            nc.sync.dma_start(cs_out[:], cs_sb[:])

    nc.compile()
    return nc


def _prepare_inputs(z1, z2):
    z = np.concatenate([np.asarray(z1), np.asarray(z2)], axis=0).astype(np.float32)
    norms = np.maximum(np.sqrt((z.astype(np.float64) ** 2).sum(-1)), EPS)
    zn = (z / norms[:, None]).astype(np.float32)
    q = np.clip(zn * FP8_SCALE, -240.0, 240.0).astype(ml_dtypes.float8_e4m3)
    # paired layout: znt_p[p, i, j] = q[j, 128*i + p]
    znt_p = np.ascontiguousarray(q.T.reshape(2, 128, TWO_N).transpose(1, 0, 2))

    ey = np.eye(128, dtype=np.float32)
    eyepair = np.concatenate(
        [(16.0 * ey), (-240.0 * ey)], axis=1
    ).astype(ml_dtypes.float8_e4m3)
    ey = ey.astype(ml_dtypes.bfloat16)
    sel = np.zeros((128, 128), dtype=ml_dtypes.bfloat16)
    for k in range(1, 9):
        sel[:, 16 * (k - 1) + k] = 1.0

    in_maps = []
    for c in range(N_CORES):
        znt_c = np.ascontiguousarray(
            np.roll(znt_p, -c * ROWS_PER_CORE, axis=2)[:, :, :SPAN]
        )
        in_maps.append({"znt": znt_c, "eyepair": eyepair, "eye": ey, "sel": sel})
    return in_maps


def kernel(z1, z2):
    if "nc" not in _cached:
        _cached["nc"] = _build_bass()
    nc = _cached["nc"]
    in_maps = _prepare_inputs(z1, z2)
    res = run_bass_kernel_spmd(nc, in_maps, core_ids=list(range(N_CORES)))
    results = res.results

    denom = np.zeros(TWO_N, dtype=np.float64)
    pexp = np.zeros(TWO_N, dtype=np.float64)
    for c in range(N_CORES):
        acc = np.asarray(results[c]["acc"], dtype=np.float64)    # [128, 25]
        px = np.asarray(results[c]["pexp"], dtype=np.float64)    # [128, 8]
        cs = np.asarray(results[c]["cs"], dtype=np.float64)      # [16, 512]
        rows = slice(c * ROWS_PER_CORE, (c + 1) * ROWS_PER_CORE)
        # row r = 128*m + p: m=0 owns acc cols 0:4, m>0 cols 3m+1:3m+4
        per_m = np.stack(
            [acc[:, 0:4].sum(-1)]
            + [acc[:, 3 * m + 1:3 * m + 4].sum(-1) for m in range(1, M_TILES)],
            axis=1,
        )  # [128, 8]
        denom[rows] += per_m.T.reshape(-1)
        pexp[rows] = px.T.reshape(-1)
        for k in range(1, 9):
            g0 = ((2 * c + k) % 16) * STRIP
            denom[g0:g0 + STRIP] += cs[k]
    loss_rows = np.log(denom) - np.log(pexp)
    return np.float32(loss_rows.mean())
